# revision 9
# baseline (speedup 1.0000x reference)
"""Trainium2 Bass kernel for causal self-attention (B=2, T=2048, C=2048, 16 heads).

Sharding: 8 cores; core c handles batch b = c // 4 and the 4 heads
h0 = (c % 4) * 4 .. h0+3.  Every (b, head) pair is fully independent,
including the final projection, because the reference's transpose-reshape maps
head h's attention output transposed into rows [h*128, (h+1)*128) of a
(T x C) matrix that then multiplies Wp^T over the *time* axis.

All matmul operands are bf16 (fp32 PSUM accumulation; measured end-to-end
rel err ~6e-3 vs the 2e-2 gate).  bf16 halves DMA/SBUF vs the fp32r
baseline and avoids the fp32r 4-cycles/row penalty on moving dims < 256.

Per-core structure:
  A: x^T resident as 16 [128,2048] tiles (4 KB DMA lines).  Q^T/K^T via
     8-accumulator stages (j-pair x 4 t-blocks, cs-major) so the first,
     DMA-paced stage consumes each (x, W) tile pair as it lands.  V
     natural via x^T-slice stationaries.
  B: flash-style per (head, 512-query block): S^T = K^T_js.T @ Q^T_blk
     with S emitted 2 js ahead of consumption so the PE never waits on
     the ACT exp; causal mask added on diagonal 128-strips; O^T and
     rowsums accumulate over js.  Tail: rowsums are PE-transposed to
     [128,4] so the reciprocal runs on 128 DVE lanes (the [1,512]
     reciprocal was 3.3 us); O^T is transposed to O natural and scaled
     by 1/rowsum per-partition during the PSUM->SBUF drain.  Tail
     transposes are deferred past the next block's first S matmuls to
     keep the PE queue fed.
  C: Y_h = O_h.T @ Wp^T with Wp^T column-blocks resident in the SBUF
     slots vacated by x^T (prefetched during B).  C groups for head h-1
     are interleaved between B blocks of head h, giving ACT slack to
     run ahead on exps.

PSUM: 8 banks as 8 single-buffer tags (a0,a1,s0,s1,s2,o,tr,rs); phase A
stages use all 8 as accumulators, phase B/C reuse them per role.
"""

import numpy as np
import ml_dtypes

import concourse.bacc as bacc
import concourse.bass as bass
import concourse.tile as tile
from concourse import mybir
from concourse.bass_utils import run_bass_kernel_spmd
from concourse.masks import make_identity

F32 = mybir.dt.float32
F32R = mybir.dt.float32r
BF16 = mybir.dt.bfloat16

B, T, C, H, HD = 2, 2048, 2048, 16, 128
P = 128
NCS = C // P        # 16 contraction subtiles for projections
NTS = T // P        # 16 t-subtiles
NTB = T // 512      # 4 moving blocks of 512
HPC = 4             # heads per core
NCORES = 8
SCALE = 1.0 / float(np.sqrt(HD))
NEG = -1.0e30

VTAGS = ["sp0", "sp1", "o", "tr", "rs", "a0"]


def build_program(reps=1, with_biases=True):
    nc = bacc.Bacc(
        "TRN2",
        target_bir_lowering=False,
        debug=False,
        enable_asserts=True,
        num_devices=NCORES,
    )

    xT = nc.dram_tensor("xT", [C, T], BF16, kind="ExternalInput").ap()
    # Q quad then K quad: [g, c, 4, d] = W[h-slice].T for 4 heads
    wqkq = nc.dram_tensor("wqkq", [2, C, HPC, HD], BF16, kind="ExternalInput").ap()
    wvT = nc.dram_tensor("wvT", [C, HPC * HD], BF16, kind="ExternalInput").ap()
    wpT = nc.dram_tensor("wpT", [T, C], BF16, kind="ExternalInput").ap()
    cmask = nc.dram_tensor("cmask", [P, P], BF16, kind="ExternalInput").ap()
    ones_d = nc.dram_tensor("ones_d", [P, 1], BF16, kind="ExternalInput").ap()
    if with_biases:
        bqs = nc.dram_tensor("bqs", [HPC * HD], F32, kind="ExternalInput").ap()
        bks = nc.dram_tensor("bks", [HPC * HD], F32, kind="ExternalInput").ap()
        bvs = nc.dram_tensor("bvs", [HPC * HD], BF16, kind="ExternalInput").ap()
        bp = nc.dram_tensor("bp", [C], BF16, kind="ExternalInput").ap()
        ones_rd = nc.dram_tensor("ones_rd", [1, P], BF16, kind="ExternalInput").ap()
    y = nc.dram_tensor("y", [HPC * HD, C], F32, kind="ExternalOutput").ap()

    with tile.TileContext(nc) as tc:
        with (
            tc.tile_pool(name="const", bufs=1) as cpool,
            tc.tile_pool(name="xsb", bufs=1) as xpool,
            tc.tile_pool(name="wqk", bufs=1) as wqpool,
            tc.tile_pool(name="wv", bufs=1) as wvpool,
            tc.tile_pool(name="qk", bufs=1) as qkpool,
            tc.tile_pool(name="vall", bufs=1) as vpool,
            tc.tile_pool(name="onat", bufs=1) as opool,
            tc.tile_pool(name="pt", bufs=1) as ptpool,
            tc.tile_pool(name="ot", bufs=1) as otpool,
            tc.tile_pool(name="small", bufs=2) as spool,
            tc.tile_pool(name="yb", bufs=2) as ypool,
            tc.tile_pool(name="ps", bufs=1, space="PSUM") as pspool,
        ):
            # ---- constants ----
            identity = cpool.tile([P, P], F32)
            make_identity(nc, identity[:])
            identity_bf = cpool.tile([P, P], BF16)
            nc.vector.tensor_copy(identity_bf[:], identity[:])
            ones_col = cpool.tile([P, 1], BF16)
            nc.sync.dma_start(ones_col[:], ones_d[:])
            cm = cpool.tile([P, P], BF16)
            nc.sync.dma_start(cm[:], cmask[:])
            # warm the ACT exp table during phase A so the first real exp
            # doesn't pay the ~1.3us ACT_TABLE_LOAD on the critical path
            warm = cpool.tile([1, 1], F32)
            nc.vector.memset(warm[:], 0.0)
            warm_o = cpool.tile([1, 1], BF16)
            nc.scalar.activation(
                warm_o[:], warm[:], mybir.ActivationFunctionType.Exp, scale=1.0
            )
            if with_biases:
                bq_sb = cpool.tile([P, HPC], F32)
                nc.sync.dma_start(bq_sb[:], bqs.rearrange("(h p) -> p h", p=P))
                bk_sb = cpool.tile([P, HPC], F32)
                nc.sync.dma_start(bk_sb[:], bks.rearrange("(h p) -> p h", p=P))
                bv_row = cpool.tile([1, HPC * HD], BF16)
                nc.sync.dma_start(bv_row[:], bvs[None, :])
                bp_sb = cpool.tile([1, C], BF16)
                nc.sync.dma_start(bp_sb[:], bp[None, :])
                ones_row = cpool.tile([1, P], BF16)
                nc.sync.dma_start(ones_row[:], ones_rd[:])

            for _rep in range(reps):
                # ---- input DMAs, arrival-ordered: (x, wq) pairs, wk, wv ----
                xsb = []
                wq = {}
                for cs in range(NCS):
                    xt = xpool.tile([P, T], BF16, tag=f"x{cs}", name=f"xsb{cs}")
                    w0 = wqpool.tile([P, HPC, HD], BF16, tag=f"w0_{cs}", name="w0")
                    if cs == 0:
                        nc.sync.dma_start(w0[:], wqkq[0, cs * P:(cs + 1) * P])
                    if cs < 2:
                        # chunked so the first matmul can start ~7us earlier
                        for tb in range(NTB):
                            nc.sync.dma_start(
                                xt[:, tb * 512:(tb + 1) * 512],
                                xT[cs * P:(cs + 1) * P, tb * 512:(tb + 1) * 512],
                            )
                    else:
                        nc.sync.dma_start(xt[:], xT[cs * P:(cs + 1) * P, :])
                    if cs != 0:
                        nc.sync.dma_start(w0[:], wqkq[0, cs * P:(cs + 1) * P])
                    xsb.append(xt)
                    wq[(0, cs)] = w0
                for cs in range(NCS):
                    w1 = wqpool.tile([P, HPC, HD], BF16, tag=f"w1_{cs}", name="w1")
                    nc.sync.dma_start(w1[:], wqkq[1, cs * P:(cs + 1) * P])
                    wq[(1, cs)] = w1
                wv_sb = []
                for cs in range(NCS):
                    wv_t = wvpool.tile([P, HPC * HD], BF16, tag=f"wv{cs}", name="wv")
                    nc.sync.dma_start(wv_t[:], wvT[cs * P:(cs + 1) * P, :])
                    wv_sb.append(wv_t)

                qTh = [
                    qkpool.tile([P, T], BF16, tag=f"qT{j}", name=f"qTh{j}")
                    for j in range(HPC)
                ]
                kTh = [
                    qkpool.tile([P, T], BF16, tag=f"kT{j}", name=f"kTh{j}")
                    for j in range(HPC)
                ]
                v_all = vpool.tile([P, NTS, HPC * HD], BF16, tag="v", name="v_all")

                # ---- phase A: projections ----
                def qk_stage(g, jpair):
                    js2 = (2 * jpair, 2 * jpair + 1)
                    keys = [(j, tb) for j in js2 for tb in range(NTB)]
                    accs = {}
                    for pi, ptag in enumerate(("sp0", "sp1")):
                        pair = pspool.tile([P, 2, 512], F32, tag=ptag, name="acc_qkp")
                        accs[keys[2 * pi]] = pair[:, 0, :]
                        accs[keys[2 * pi + 1]] = pair[:, 1, :]
                    for si, stag in enumerate(("o", "tr", "rs", "a0")):
                        accs[keys[4 + si]] = pspool.tile(
                            [P, 512], F32, tag=stag, name="acc_qk"
                        )[:]
                    for cs in range(NCS):
                        for j, tb in keys:
                            nc.tensor.matmul(
                                accs[(j, tb)], wq[(g, cs)][:, j, :],
                                xsb[cs][:, tb * 512:(tb + 1) * 512],
                                start=(cs == 0), stop=(cs == NCS - 1),
                            )
                    dsts = qTh if g == 0 else kTh
                    for j, tb in keys:
                        dst = dsts[j][:, tb * 512:(tb + 1) * 512]
                        if with_biases:
                            b_sb = bq_sb if g == 0 else bk_sb
                            nc.vector.tensor_tensor(
                                dst, accs[(j, tb)],
                                b_sb[:, j, None].to_broadcast([P, 512]),
                                mybir.AluOpType.add,
                            )
                        else:
                            nc.vector.tensor_copy(dst, accs[(j, tb)])

                qk_stage(0, 0)   # Q heads 0,1  (DMA-paced)
                qk_stage(1, 0)   # K heads 0,1
                # V natural: [t, d-cat]
                for ts in range(NTS):
                    acc = pspool.tile([P, 512], F32, tag=VTAGS[ts % 6], name="acc_v")
                    for cs in range(NCS):
                        nc.tensor.matmul(
                            acc[:], xsb[cs][:, ts * P:(ts + 1) * P], wv_sb[cs][:],
                            start=(cs == 0),
                            stop=(cs == NCS - 1 and not with_biases),
                        )
                    if with_biases:
                        nc.tensor.matmul(
                            acc[:], ones_row[:], bv_row[:], start=False, stop=True
                        )
                    nc.vector.tensor_copy(v_all[:, ts, :], acc[:])
                qk_stage(0, 1)   # Q heads 2,3
                qk_stage(1, 1)   # K heads 2,3

                # ---- Wp^T prefetch into the x slots (consumed by phase C) ----
                wp_sb = []
                for ts in range(NTS):
                    wpt = xpool.tile([P, T], BF16, tag=f"x{ts}", name=f"wp{ts}")
                    nc.sync.dma_start(wpt[:], wpT[ts * P:(ts + 1) * P, :])
                    wp_sb.append(wpt)

                o_nat = [
                    opool.tile([P, NTS, HD], BF16, tag=f"on{h}", name=f"onat{h}")
                    for h in range(HPC)
                ]

                # ---- phases B (attention) and C (output proj), interleaved ----
                pending_tail = []

                def flush_tail():
                    while pending_tail:
                        h, ib, oT, rs_row = pending_tail.pop(0)
                        ps_rst = pspool.tile([P, 4], F32, tag="tr", name="ps_rst")
                        for tch in range(4):
                            nc.tensor.transpose(
                                ps_rst[:, tch:tch + 1],
                                rs_row[:, tch * P:(tch + 1) * P],
                                identity[0:1, 0:1],
                            )
                        rs_rec = spool.tile([P, 4], F32, tag="rsrec", name="rs_rec")
                        nc.vector.reciprocal(rs_rec[:], ps_rst[:])
                        ps_tr = pspool.tile([P, 4, HD], BF16, tag="tr", name="ps_tr")
                        for tch in range(4):
                            nc.tensor.transpose(
                                ps_tr[:, tch, :], oT[:, tch * P:(tch + 1) * P],
                                identity_bf[:],
                            )
                        nc.vector.tensor_tensor(
                            o_nat[h][:, ib * 4:(ib + 1) * 4, :], ps_tr[:, :, :],
                            rs_rec[:, :, None].to_broadcast([P, 4, HD]),
                            mybir.AluOpType.mult,
                        )

                def emit_B(h, ib):
                    njs = 4 * ib + 4
                    npair = njs // 2
                    i0 = ib * 512
                    ps_o = pspool.tile([P, 512], F32, tag="o", name="ps_o")
                    ps_rs = pspool.tile([1, 512], F32, tag="rs", name="ps_rs")
                    ptiles = {}

                    def emit_S_pair(k):
                        ps2 = pspool.tile(
                            [P, 2, 512], F32, tag=f"sp{k % 2}", name="ps_s2"
                        )
                        c0s = []
                        for jj in range(2):
                            js = 2 * k + jj
                            r = js - 4 * ib
                            c0 = max(0, r) * P
                            c0s.append(c0)
                            nc.tensor.matmul(
                                ps2[:, jj, c0:],
                                kTh[h][:, js * P:(js + 1) * P],
                                qTh[h][:, i0 + c0:i0 + 512],
                                start=True, stop=True,
                            )
                            if c0 > 0:
                                # cols left of the diagonal strip are never
                                # written: set to -inf so the pair-wide exp
                                # maps them to 0
                                nc.vector.memset(ps2[:, jj, 0:c0], NEG)
                            if r >= 0:
                                nc.vector.tensor_tensor(
                                    ps2[:, jj, c0:c0 + P], ps2[:, jj, c0:c0 + P],
                                    cm[:], mybir.AluOpType.add,
                                )
                        # one exp + one semaphore for the whole pair
                        pt2 = ptpool.tile(
                            [P, 2, 512], BF16, tag=f"pt{k % 2}", name="pt2"
                        )
                        nc.scalar.activation(
                            pt2[:, :, :], ps2[:, :, :],
                            mybir.ActivationFunctionType.Exp,
                            scale=SCALE,
                        )
                        ptiles[k] = (pt2, c0s)

                    emit_S_pair(0)
                    flush_tail()
                    for k in range(npair):
                        if k + 1 < npair:
                            emit_S_pair(k + 1)
                        pt2, c0s = ptiles.pop(k)
                        for jj in range(2):
                            js = 2 * k + jj
                            nc.tensor.matmul(
                                ps_o[:, c0s[jj]:],
                                v_all[:, js, h * HD:(h + 1) * HD],
                                pt2[:, jj, c0s[jj]:],
                                start=(js == 0), stop=(js == njs - 1),
                            )
                        for jj in range(2):
                            js = 2 * k + jj
                            nc.tensor.matmul(
                                ps_rs[:, c0s[jj]:], ones_col[:],
                                pt2[:, jj, c0s[jj]:],
                                start=(js == 0), stop=(js == njs - 1),
                            )
                    oT = otpool.tile([P, 512], BF16, tag=f"ot{ib % 2}", name="oT")
                    nc.vector.tensor_copy(oT[:], ps_o[:])
                    rs_row = spool.tile([1, 512], F32, tag="rsrow", name="rs_row")
                    nc.vector.tensor_copy(rs_row[:], ps_rs[:])
                    pending_tail.append((h, ib, oT, rs_row))

                def emit_C(h, jb):
                    acc = pspool.tile([P, 512], F32, tag="a0", name="acc_c")
                    for ts in range(NTS):
                        nc.tensor.matmul(
                            acc[:], o_nat[h][:, ts, :],
                            wp_sb[ts][:, jb * 512:(jb + 1) * 512],
                            start=(ts == 0),
                            stop=(ts == NTS - 1 and not with_biases),
                        )
                    if with_biases:
                        nc.tensor.matmul(
                            acc[:], ones_row[:],
                            bp_sb[:, jb * 512:(jb + 1) * 512],
                            start=False, stop=True,
                        )
                    yb = ypool.tile([P, 512], F32, tag="yb")
                    nc.vector.tensor_copy(yb[:], acc[:])
                    nc.sync.dma_start(
                        y[h * HD:(h + 1) * HD, jb * 512:(jb + 1) * 512], yb[:]
                    )

                for h in range(HPC):
                    for ib in range(NTB):
                        emit_B(h, ib)
                        if h >= 1:
                            emit_C(h - 1, ib)
                flush_tail()
                for jb in range(NTB):
                    emit_C(HPC - 1, jb)

    nc.compile()
    return nc


def make_in_maps(x, Wq, bq, Wk, bk, Wv, bv, Wp, bp):
    BF = ml_dtypes.bfloat16
    x = np.asarray(x, dtype=np.float32)
    wpT = np.ascontiguousarray(
        np.asarray(Wp, dtype=np.float32).T.astype(BF)
    )
    f = np.arange(P, dtype=np.int64)[None, :]
    p = np.arange(P, dtype=np.int64)[:, None]
    cmask = np.where(f >= p, 0.0, NEG).astype(BF)

    xTs = [np.ascontiguousarray(x[b].T.astype(BF)) for b in range(B)]
    any_bias = any(np.any(np.asarray(b)) for b in (bq, bk, bv, bp))
    in_maps = []
    for core in range(NCORES):
        b = core // 4
        h0 = (core % 4) * HPC
        hsl = slice(h0 * HD, (h0 + HPC) * HD)

        def wt3(W):
            # (HPC, C, HD) contiguous: per-head [c, d] transposed weight
            ws = np.asarray(W, dtype=np.float32)[hsl].T  # (C, HPC*HD)
            return np.ascontiguousarray(ws.reshape(C, HPC, HD).transpose(1, 0, 2))

        # quads: (2, C, 4, HD) — all 4 heads of Q (then K) per c-row
        wqkq = np.ascontiguousarray(np.stack([
            wt3(Wq).transpose(1, 0, 2), wt3(Wk).transpose(1, 0, 2)
        ]).astype(BF))  # (2, C, 4, HD)
        wvT = np.ascontiguousarray(
            np.asarray(Wv, np.float32)[hsl].T.astype(BF)
        )  # (C, 512)

        im = {
            "xT": xTs[b],
            "wqkq": wqkq,
            "wvT": wvT,
            "wpT": wpT,
            "ones_d": np.ones((P, 1), dtype=BF),
            "cmask": cmask,
        }
        if any_bias:
            im["bqs"] = np.ascontiguousarray(np.asarray(bq, np.float32)[hsl])
            im["bks"] = np.ascontiguousarray(np.asarray(bk, np.float32)[hsl])
            im["bvs"] = np.ascontiguousarray(
                np.asarray(bv, np.float32)[hsl].astype(BF))
            im["bp"] = np.asarray(bp, dtype=np.float32).astype(BF)
            im["ones_rd"] = np.ones((1, P), dtype=BF)
        in_maps.append(im)
    return in_maps


_NC_CACHE = {}


def get_nc(with_biases=False):
    return get_nc_reps(1, with_biases)


def get_nc_reps(reps, with_biases=False):
    key = (reps, with_biases)
    if key not in _NC_CACHE:
        _NC_CACHE[key] = build_program(reps, with_biases)
    return _NC_CACHE[key]


def assemble(results):
    out = np.empty((B, T, C), dtype=np.float32)
    for core in range(NCORES):
        b = core // 4
        h0 = (core % 4) * HPC
        out[b, h0 * HD:(h0 + HPC) * HD, :] = results[core]["y"]
    return out


def kernel(x, Wq, bq, Wk, bk, Wv, bv, Wp, bp):
    any_bias = any(
        np.any(np.asarray(b)) for b in (bq, bk, bv, bp)
    )
    nc = get_nc(with_biases=bool(any_bias))
    in_maps = make_in_maps(x, Wq, bq, Wk, bk, Wv, bv, Wp, bp)
    res = run_bass_kernel_spmd(nc, in_maps, list(range(NCORES)))
    return assemble(res.results)
